# revision 12
# baseline (speedup 1.0000x reference)
"""Trainium2 Bass kernel for nn_Capsule (dynamic routing, 3 iterations).

Reference computation (per batch b, hidden h, routing dim r=64):
  v0 = squash(mean_r x)                      squash(s) = s * ||s||/(1+||s||)
  for u in (v0, v0+v1):
      w   = softmax_r(x * u)                 (softmax over r, per (b,h))
      s   = sum_r w * x
      v   = squash(s)
  return v2                                  shape [B, H]

Sharding: pure data parallel over batch across 8 NeuronCores.

v2 design (vs fp32 baseline): the whole elementwise chain runs in bf16.
  - x is cast fp32->bf16 *during* the HBM->SBUF DMA (SWDGE cast, free)
  - logits = x*u, prod = e*x   -> VectorE bf16 tensor_tensor (2x mode),
                                  a slice of prods on GpSimd to balance
  - e = exp(logits)            -> ScalarE (rate is dtype-independent)
  - sum_r reductions           -> TensorE identity-bf16 matmuls into PSUM
                                  (fp32 accumulate keeps the sums accurate)
  - squash / divide            -> VectorE fp32 (bit-hack rsqrt + NR,
                                  reciprocal_approx_fast for 1/den)
Emission is software-pipelined one chunk deep so VectorE does not stall
on ScalarE's exp latency.
"""

import numpy as np

B, R, H = 2048, 64, 512
N_CORES = 8
BPC = B // N_CORES  # batches per core
P = 128             # partitions (batches per group)

# Tunables
RT = 16             # r-slices per x DMA tile
CH = 8              # r-slices per compute chunk
PROD_G_EVERY = 3    # every Nth chunk's prod-mul runs on GpSimd, rest VectorE
                    # (0 = all prods on VectorE)
LG_G_PERIOD = 0     # every Nth chunk's logits-mul runs on GpSimd (0 = none)
XB_BUFS_EXTRA = 2   # extra x-tile buffers beyond one group's worth

_PROGRAM_CACHE = {}


def _build_program(bpc=BPC, reps=1):
    import concourse.tile as tile
    from concourse import bacc, mybir

    f32 = mybir.dt.float32
    bf16 = mybir.dt.bfloat16
    i32 = mybir.dt.int32
    AF = mybir.ActivationFunctionType
    OP = mybir.AluOpType
    AX = mybir.AxisListType

    G = bpc // P        # groups of 128 batches
    NT = R // RT        # x tiles per group
    CPT = RT // CH      # compute chunks per x tile
    NCH = R // CH       # compute chunks per group

    nc = bacc.Bacc(
        "TRN2",
        target_bir_lowering=False,
        debug=False,
        enable_asserts=False,
    )
    x_d = nc.dram_tensor("x", [bpc, R, H], f32, kind="ExternalInput").ap()
    id_d = nc.dram_tensor("ident", [P, P], f32, kind="ExternalInput").ap()
    out_d = nc.dram_tensor("out", [bpc, H], f32, kind="ExternalOutput").ap()

    with tile.TileContext(nc) as tc:
        with (
            tc.tile_pool(name="xbp", bufs=NT + XB_BUFS_EXTRA) as xbp,
            tc.tile_pool(name="lgp", bufs=3) as lgp,
            tc.tile_pool(name="epp", bufs=4) as epp,
            tc.tile_pool(name="ppp", bufs=3) as ppp,
            tc.tile_pool(name="dnp", bufs=2) as dnp,
            tc.tile_pool(name="cst", bufs=1) as cst,
            tc.tile_pool(name="outp", bufs=2) as outp,
            tc.tile_pool(name="psp", bufs=2, space="PSUM") as psp,
        ):
            identb = cst.tile([P, P], bf16)
            nc.gpsimd.dma_start(identb[:], id_d)  # cast f32 -> bf16
            magic = cst.tile([P, 1], i32)
            nc.vector.memset(magic[:], 0x5F3759DF)

            def squash_factor(s_ap, tag):
                """gsc[p,1] = sn/(1+sn) = 1/(1+rsqrt(nrm)), sn=||s||_2.

                rsqrt via bit-hack seed + 2 Newton iterations, VectorE only
                (avoids ScalarE act-table switches)."""
                sq = dnp.tile([P, H], f32, name=f"sq_{tag}", tag="sq")
                nc.scalar.activation(sq[:], s_ap, AF.Square)
                nrm = dnp.tile([P, 1], f32, name=f"nrm_{tag}", tag="nrm")
                nc.vector.reduce_sum(nrm[:], sq[:], axis=AX.X)
                half_i = dnp.tile([P, 1], i32, name=f"hi_{tag}", tag="hi")
                nc.vector.tensor_scalar(
                    half_i[:], nrm[:].bitcast(i32), 1, None,
                    op0=OP.arith_shift_right,
                )
                y0 = dnp.tile([P, 1], i32, name=f"y0_{tag}", tag="y0")
                nc.vector.scalar_tensor_tensor(
                    y0[:], magic[:], 0, half_i[:],
                    op0=OP.bypass, op1=OP.subtract,
                )
                y = y0[:].bitcast(f32)
                for nr in range(2):
                    t1 = dnp.tile([P, 1], f32, name=f"t1_{tag}_{nr}", tag="t1")
                    nc.vector.tensor_mul(t1[:], y, y)
                    t2 = dnp.tile([P, 1], f32, name=f"t2_{tag}_{nr}", tag="t2")
                    nc.vector.tensor_mul(t2[:], t1[:], nrm[:])
                    t3 = dnp.tile([P, 1], f32, name=f"t3_{tag}_{nr}", tag="t3")
                    nc.vector.tensor_scalar(
                        t3[:], t2[:], -0.5, 1.5, op0=OP.mult, op1=OP.add
                    )
                    yn = dnp.tile([P, 1], f32, name=f"y_{tag}_{nr}", tag="yn")
                    nc.vector.tensor_mul(yn[:], y, t3[:])
                    y = yn[:]
                y1 = dnp.tile([P, 1], f32, name=f"y1_{tag}", tag="y1")
                nc.vector.tensor_scalar_add(y1[:], y, 1.0)
                gsc = dnp.tile([P, 1], f32, name=f"gsc_{tag}", tag="gsc")
                nc.vector.reciprocal(gsc[:], y1[:])
                return gsc

            for rep, g in [(r_, g_) for r_ in range(reps) for g_ in range(G)]:
                xg = x_d[g * P:(g + 1) * P]  # [128, R, H] fp32 in HBM
                xt = []
                for t in range(NT):
                    x_t = xbp.tile([P, RT, H], bf16, name="xtile", tag="xtile")
                    # SWDGE cast DMA: fp32 HBM -> bf16 SBUF
                    nc.gpsimd.dma_start(
                        x_t[:], xg[:, t * RT:(t + 1) * RT, :]
                    )
                    xt.append(x_t)

                def xs_of(ci):
                    t, c = divmod(ci, CPT)
                    return xt[t][:, c * CH:(c + 1) * CH, :]

                # ---- iter 0: mean over r via identity-matmul accumulation
                mean_ps = psp.tile([P, H], f32, name=f"mean_{g}", tag="mean")
                for t in range(NT):
                    for r in range(RT):
                        nc.tensor.matmul(
                            mean_ps[:],
                            identb[:],
                            xt[t][:, r, :],
                            start=(t == 0 and r == 0),
                            stop=(t == NT - 1 and r == RT - 1),
                        )
                s0 = dnp.tile([P, H], f32, name=f"s0_{g}", tag="s0")
                nc.scalar.mul(s0[:], mean_ps[:], 1.0 / R)
                gsc0 = squash_factor(s0[:], f"{g}_0")
                v0 = dnp.tile([P, H], f32, name=f"v0_{g}", tag="v0")
                nc.scalar.mul(v0[:], s0[:], gsc0[:])
                u = dnp.tile([P, H], bf16, name=f"u_{g}", tag="u")
                nc.scalar.mul(u[:], s0[:], gsc0[:])

                # ---- iters 1, 2 (software-pipelined one chunk deep)
                for it in (1, 2):
                    den_ps = psp.tile([P, H], f32, name=f"den_{g}_{it}", tag="den")
                    num_ps = psp.tile([P, H], f32, name=f"num_{g}_{it}", tag="num")
                    ub = (
                        u[:]
                        .rearrange("p (a h) -> p a h", a=1)
                        .broadcast_to([P, CH, H])
                    )
                    eps = [None] * NCH
                    for ci in range(NCH + 1):
                        if ci < NCH:
                            lg = lgp.tile([P, CH, H], bf16, name="lg", tag="lg")
                            if LG_G_PERIOD and ci % LG_G_PERIOD == LG_G_PERIOD - 1:
                                nc.gpsimd.tensor_tensor(
                                    lg[:], xs_of(ci), ub, op=OP.mult
                                )
                            else:
                                nc.vector.tensor_tensor(
                                    lg[:], xs_of(ci), ub, op=OP.mult
                                )
                            ep = epp.tile([P, CH, H], bf16, name="ep", tag="ep")
                            nc.scalar.activation(ep[:], lg[:], AF.Exp)
                            eps[ci] = ep
                        if ci > 0:
                            pi = ci - 1
                            epv = eps[pi]
                            eps[pi] = None
                            pp = ppp.tile([P, CH, H], bf16, name="pp", tag="pp")
                            if PROD_G_EVERY and pi % PROD_G_EVERY == PROD_G_EVERY - 1:
                                nc.gpsimd.tensor_tensor(
                                    pp[:], epv[:], xs_of(pi), op=OP.mult
                                )
                            else:
                                nc.vector.tensor_tensor(
                                    pp[:], epv[:], xs_of(pi), op=OP.mult
                                )
                            for r in range(CH):
                                nc.tensor.matmul(
                                    den_ps[:],
                                    identb[:],
                                    epv[:, r, :],
                                    start=(pi == 0 and r == 0),
                                    stop=(pi == NCH - 1 and r == CH - 1),
                                )
                            for r in range(CH):
                                nc.tensor.matmul(
                                    num_ps[:],
                                    identb[:],
                                    pp[:, r, :],
                                    start=(pi == 0 and r == 0),
                                    stop=(pi == NCH - 1 and r == CH - 1),
                                )
                    rd = dnp.tile([P, H], f32, name=f"rd_{g}_{it}", tag="rd")
                    nc.vector.reciprocal_approx_fast(rd[:], den_ps[:])
                    s = dnp.tile([P, H], f32, name=f"s_{g}_{it}", tag="s")
                    nc.vector.tensor_mul(s[:], num_ps[:], rd[:])
                    gsc = squash_factor(s[:], f"{g}_{it}")
                    if it == 1:
                        u2 = dnp.tile([P, H], bf16, name=f"u2_{g}", tag="u2")
                        # u2 = s*gsc + v0
                        nc.vector.scalar_tensor_tensor(
                            u2[:], s[:], gsc[:], v0[:], op0=OP.mult, op1=OP.add
                        )
                        u = u2
                    else:
                        o = outp.tile([P, H], f32, name="o", tag="o")
                        nc.scalar.mul(o[:], s[:], gsc[:])
                        nc.sync.dma_start(out_d[g * P:(g + 1) * P, :], o[:])

    nc.compile()
    return nc


def _get_program(bpc=BPC, reps=1):
    key = (bpc, reps)
    if key not in _PROGRAM_CACHE:
        _PROGRAM_CACHE[key] = _build_program(bpc, reps)
    return _PROGRAM_CACHE[key]


def _identity_np():
    return np.eye(P, dtype=np.float32)


def kernel(input_matrix: np.ndarray) -> np.ndarray:
    from concourse.bass_utils import run_bass_kernel_spmd

    x = np.ascontiguousarray(np.asarray(input_matrix, dtype=np.float32))
    assert x.shape == (B, R, H)
    nc = _get_program()
    ident = _identity_np()
    shards = x.reshape(N_CORES, BPC, R, H)
    in_maps = [
        {"x": np.ascontiguousarray(shards[i]), "ident": ident}
        for i in range(N_CORES)
    ]
    res = run_bass_kernel_spmd(nc, in_maps, core_ids=list(range(N_CORES)))
    out = np.concatenate(
        [res.results[i]["out"] for i in range(N_CORES)], axis=0
    )
    return out


if __name__ == "__main__":
    nc = _get_program()
    print("program built and compiled OK")
